# revision 66
# baseline (speedup 1.0000x reference)
"""HSMNet cost-volume + disparity softmax-regression on 8 Trainium2 NeuronCores.

Reference computation (per batch b):
  cost[c,d,h,w] = |ref[c,h,w] - tgt[c,h,w-d]| for w>=d else 0
  cost_agg[d,h,w] = sum_c cost
  pred[h,w] = sum_d d * softmax_d(cost_agg)

Sharding: 8 cores = 4 batches x 2 h-halves (40 rows of 80 each). Each core
processes its [32, 40, 160] slice fully fused on-chip.

Host prep (layout only, no arithmetic): inputs are cast to fp16 and
replicated into 4 partition groups (partition = c + 32*j) with the shift j
baked into tgt via a 24-col front zero pad. On-chip, per eighth of the
pixel range (800 pixels):
  - one DVE tensor_tensor subtract with a 3D access pattern (disparity
    block dim stride +4 on tgt, stride 0 broadcast on ref) produces diffs
    for all 24 disparities: diff[c+32j, k, p] = ref[c,p] - tgt[c, p-4b-j],
    b = 5-k.
  - abs in place, split across DVE (uint16 bitand), ACT (Abs), GPSIMD
    (uint16 bitand) per env-tunable column split.
  - TensorE reduces channels with 0/1 weights into PSUM [24, 2x512], plus
    one extra accumulation matmul that adds -10000 where w < d (validity
    mask folded into the PE pass: [w<d] = sum_k [k<d]*[w==k]).
  - ACT Exp evacuates PSUM -> E[96, 1600] bf16 (rows 24q+d).
  - TensorE contracts E with [ones; d] weights -> den/num [8, 1600].
  - host divides num/den (invalid entries' terms vanish: exp(-1e4) = 0).
"""
import os
import sys
import threading

for _p in ("/opt/trn_rl_repo",):
    if os.path.isdir(_p) and _p not in sys.path:
        sys.path.insert(0, _p)

import numpy as np
import ml_dtypes

import concourse.bacc as bacc
import concourse.mybir as mybir
from concourse.tile import TileContext
from concourse.bass_utils import run_bass_kernel_spmd

dt = mybir.dt

# problem shape (hardcoded per spec)
B, C, H, W = 4, 32, 80, 160
D = 24
HP = H // 2            # rows per core
PIX = HP * W           # 6400 pixels per core
NB = D // 4            # 6 disparity blocks of 4
PAD = 24               # zero pad columns in front of tgtr
NE = 8                 # processing units (eighths of the pixel range)
EW = PIX // NE         # 800 pixels per eighth
QW = PIX // 4          # 1600 pixels per quarter (E column range)
N_CORES = 8

# abs column split within each [128, 6*w] diff tile: [0:A) DVE bitand,
# [A:) ACT Abs (scaled by unit width). GPSIMD compute is NOT used: it
# shares an SBUF port with the DVE and degrades DVE throughput ~20%.
ABS_DVE = int(os.environ.get("HSM_ABS_DVE", "2688"))
DIFF_BUFS = int(os.environ.get("HSM_DIFF_BUFS", "5"))
COST_BUFS = int(os.environ.get("HSM_COST_BUFS", "2"))

# processing units (pixel offset, width): sixteenths at the start (early
# first subtract off a small first DMA) and at the end (short final
# drain abs->PE->exp->nd->copy->out); eighths in the middle. The last
# TAIL_FULL units run abs fully on DVE so ACT owes nothing at the end.
UNITS = [(0, 400), (400, 400)] + \
        [(EW * e, EW) for e in range(1, NE - 1)] + \
        [(EW * (NE - 1), 400), (EW * (NE - 1) + 400, 400)]
NU = len(UNITS)
TAIL_FULL = int(os.environ.get("HSM_TAIL_FULL", "2"))
# per-unit packed input block: [ref_w | tgt_(PAD+w)], width 2w+PAD
BLK_OFF = []
_o = 0
for (_p0, _w) in UNITS:
    BLK_OFF.append(_o)
    _o += 2 * _w + PAD
IN_TOTAL = _o


# packed consts (all fp16 container): lred | lnd(bf16 bits) | lmask | maskc
CONST_W = NB * D + 8 + D + EW  # 144+8+24+800 = 976


def _build_program():
    nc = bacc.Bacc("TRN2", target_bir_lowering=False)
    inq_h = nc.dram_tensor("inq", [128, IN_TOTAL], dt.float16,
                           kind="ExternalInput")
    cst_h = nc.dram_tensor("cst", [128, CONST_W], dt.float16,
                           kind="ExternalInput")
    out_h = nc.dram_tensor("out", [8, 4 * 400], dt.float32, kind="ExternalOutput")

    with TileContext(nc) as tc:
        with tc.tile_pool(name="const", bufs=1) as cpool, \
             tc.tile_pool(name="inp", bufs=6) as ipool, \
             tc.tile_pool(name="diffp", bufs=DIFF_BUFS) as dpool, \
             tc.tile_pool(name="ep", bufs=1) as epool:
            cst_sb = cpool.tile([128, CONST_W], dt.float16)
            lred_sb = cst_sb[:, 0:NB * D]
            lnd_sb = cst_sb[:, NB * D:NB * D + 8].bitcast(dt.bfloat16)
            lmask_sb = cst_sb[0:D, NB * D + 8:NB * D + 8 + D]
            maskc_sb = cst_sb[0:D, NB * D + 8 + D:CONST_W]

            E = epool.tile([128, QW], dt.bfloat16)

            # one packed DMA per unit (ref_w | shifted tgt window), spread
            # over the sync/scalar trigger queues so the ~0.6us
            # descriptor-gen per DMA doesn't serialize ahead of unit 0
            inq = {}

            def emit_load(u, eng):
                _, w = UNITS[u]
                bw = 2 * w + PAD
                t_ = ipool.tile([128, 2 * EW + PAD], dt.float16, tag="inq",
                                name=f"inq_{u}", bufs=6)
                eng.dma_start(t_[:, 0:bw],
                              inq_h[:, BLK_OFF[u]:BLK_OFF[u] + bw])
                inq[u] = t_

            emit_load(0, nc.sync)
            emit_load(1, nc.scalar)
            emit_load(2, nc.sync)
            emit_load(3, nc.scalar)
            nc.sync.dma_start(cst_sb[:], cst_h[:])
            # rows 32q+24..32q+31 are never written by the exp evac; zero
            # them once so the num/den matmul sees 0 (their lnd weights are
            # 0, but garbage could be inf/nan)
            nc.gpsimd.memset(E[:], 0.0)
            for u in range(4, NU):
                emit_load(u, nc.sync if u % 2 == 0 else nc.scalar)

            diffs = {}

            def emit_tt(u):
                _, w = UNITS[u]
                diff = dpool.tile([128, NB * EW], dt.float16, tag="diff",
                                  name=f"diff_{u}", bufs=DIFF_BUFS)
                out = diff[:, 0:NB * w].rearrange("p (k x) -> p k x", x=w)
                in0 = inq[u][:, 0:w].unsqueeze(1).broadcast_to(
                    [128, NB, w])
                # tgt part starts at col w; block k reads window cols
                # [w+4+4k, +w) -> diff slot k holds block b = 5-k
                in1 = inq[u][:, w + 4:w + 4 + w].unsqueeze(
                    1).broadcast_to([128, NB, w]).copy()
                in1.ap = in1.ap[:1] + (((4, NB)),) + in1.ap[2:]
                nc.vector.tensor_tensor(out, in0, in1,
                                        mybir.AluOpType.subtract)
                diffs[u] = diff
                # tail units have no ACT abs; run their (full-width) DVE
                # bitand right away so their PE/exp overlap the next TT
                if u >= NU - TAIL_FULL:
                    emit_abs_dve(u)
                    abs_done.add(u)

            def _asplit(u):
                w = UNITS[u][1]
                if u >= NU - TAIL_FULL:
                    return NB * w
                return ABS_DVE * w // EW

            abs_done = set()

            def emit_abs_dve(u):
                if u in abs_done:
                    return
                a = _asplit(u)
                if a > 0:
                    du = diffs[u][:, 0:a].bitcast(dt.uint16)
                    nc.vector.tensor_scalar(du, du, 0x7FFF, None,
                                            mybir.AluOpType.bitwise_and)

            def emit_abs_act(u):
                a, w = _asplit(u), UNITS[u][1]
                if a < NB * w:
                    nc.scalar.activation(diffs[u][:, a:NB * w],
                                         diffs[u][:, a:NB * w],
                                         mybir.ActivationFunctionType.Abs)

            costs = {}

            def emit_pe(u, qpool):
                p0, w = UNITS[u]
                diff = diffs[u]
                ccw = min(400, w)
                ncc = w // ccw
                cost = qpool.tile([D, 1024], dt.float32, tag="cost",
                                  name=f"cost_{u}")
                for k in range(NB):
                    b = NB - 1 - k
                    for cc in range(ncc):
                        nc.tensor.matmul(
                            cost[:, 512 * cc:512 * cc + ccw],
                            lred_sb[:, D * b:D * (b + 1)],
                            diff[:, w * k + ccw * cc:w * k + ccw * cc + ccw],
                            start=(k == 0), stop=False)
                for cc in range(ncc):
                    mo = (p0 + ccw * cc) % EW
                    nc.tensor.matmul(
                        cost[:, 512 * cc:512 * cc + ccw],
                        lmask_sb[:],
                        maskc_sb[:, mo:mo + ccw],
                        start=False, stop=(cc == ncc - 1))
                costs[u] = cost

            def emit_exp(u):
                p0, w = UNITS[u]
                q, ecol = p0 // QW, p0 % QW
                ccw = min(400, w)
                ncc = w // ccw
                cost = costs[u]
                src = cost[:, 0:512 * ncc].rearrange(
                    "p (k x) -> p k x", x=512)[:, :, 0:ccw]
                dst = E[32 * q:32 * q + D, ecol:ecol + w].rearrange(
                    "p (k x) -> p k x", x=ccw)
                nc.scalar.activation(dst, src, mybir.ActivationFunctionType.Exp)
                del costs[u], diffs[u]

            with tc.tile_pool(name="cost", bufs=COST_BUFS, space="PSUM") as qpool, \
                 tc.tile_pool(name="nd", bufs=1, space="PSUM") as npool:
                nd = npool.tile([8, 2048], dt.float32)

                # HAM warm-up: keep the PE clock at full rate until the
                # first real matmul burst (reads zeroed E, overwritten by
                # the real nd accumulation later). Must bridge the whole
                # window to the first real matmul (~15us) or HAM
                # re-throttles after ~3.4us idle.
                for _ in range(16):
                    nc.tensor.matmul(nd[:, 0:400], E[:, 0:8], E[:, 0:400],
                                     start=True, stop=True)

                def emit_nd(half):
                    # partial num/den contraction over quarters 2h, 2h+1
                    # (PE operands cannot start at partition 96, so halves)
                    for cc in range(4):
                        nc.tensor.matmul(
                            nd[:, 512 * cc:512 * cc + 400],
                            lnd_sb[64 * half:64 * half + 64, :],
                            E[64 * half:64 * half + 64,
                              400 * cc:400 * (cc + 1)],
                            start=(half == 0), stop=(half == 1))

                for u in range(NU):
                    emit_tt(u)
                    emit_abs_act(u)
                    if u >= 1:
                        emit_abs_dve(u - 1)
                        emit_pe(u - 1, qpool)
                        emit_exp(u - 1)
                        if u == 5:
                            emit_nd(0)
                emit_abs_dve(NU - 1)
                emit_pe(NU - 1, qpool)
                emit_exp(NU - 1)
                emit_nd(1)

                # evacuate nd per 512-chunk as each region's accumulation
                # completes, split across DVE (cc0/1) and ACT (cc2/3)
                ndv = nd[:].rearrange("p (k x) -> p k x", x=512)[:, :, 0:400]
                out_a = epool.tile([8, 800], dt.float32)
                out_b = epool.tile([8, 800], dt.float32)
                nc.vector.tensor_copy(
                    out_a[:].rearrange("p (k x) -> p k x", x=400),
                    ndv[:, 0:2])
                nc.scalar.activation(
                    out_b[:].rearrange("p (k x) -> p k x", x=400),
                    ndv[:, 2:4], mybir.ActivationFunctionType.Copy)
                nc.sync.dma_start(out_h[:, 0:800], out_a[:])
                nc.scalar.dma_start(out_h[:, 800:1600], out_b[:])

    nc.compile()
    return nc


def _host_constants():
    """Single packed [128, CONST_W] fp16 tensor: lred | lnd bits | lmask | maskc."""
    cst = np.zeros((128, CONST_W), np.float16)
    # lred: block b sums channels of partition group j into cost row 4b+j
    for b in range(NB):
        for j in range(4):
            for c in range(C):
                cst[c + 32 * j, D * b + 4 * b + j] = 1.0

    # lnd (bf16, stored as raw bits in the fp16 container)
    lnd = np.zeros((128, 8), np.float32)
    for q in range(4):
        for d in range(D):
            lnd[d + 32 * q, q] = 1.0      # den
            lnd[d + 32 * q, 4 + q] = d    # num
    lnd16 = lnd.astype(ml_dtypes.bfloat16).view(np.float16)
    cst[:, NB * D:NB * D + 8] = lnd16

    # bias[d, p] = sum_k lmask[k, d] * maskc[k, p] = -1e4 * [(p mod W) < d]
    o = NB * D + 8
    for k in range(D):
        for d in range(D):
            if k < d:
                cst[k, o + d] = 1.0
    o = NB * D + 8 + D
    for k in range(D):
        cst[k, o + np.arange(EW)[np.arange(EW) % W == k]] = -10000.0
    return cst


_lock = threading.Lock()
_cache = {}


def _get_program():
    with _lock:
        if "nc" not in _cache:
            _cache["nc"] = _build_program()
            _cache["consts"] = _host_constants()
        return _cache["nc"], _cache["consts"]


def _prep_core(ref_s, tgt_s):
    """ref_s, tgt_s: [32, 6400] fp16 -> packed [128, IN_TOTAL] input."""
    inq = np.zeros((128, IN_TOTAL), np.float16)
    for u, (c0, w) in enumerate(UNITS):
        o = BLK_OFF[u]
        inq[:, o:o + w] = np.broadcast_to(
            ref_s[None, :, c0:c0 + w], (4, C, w)).reshape(128, w)
        # tgt window [c0 - PAD, c0 + w) with per-group shift j baked in:
        # col o + w + s holds tgt[c, c0 - PAD + s - j]
        for j in range(4):
            lo = c0 - PAD - j
            src = tgt_s[:, max(lo, 0):c0 + w - j]
            d0 = o + w + max(lo, 0) - lo
            inq[32 * j:32 * j + 32, d0:d0 + src.shape[1]] = src
    return inq


def _run(refimg_fea, targetimg_fea, trace=False):
    nc, cst = _get_program()
    ref = np.asarray(refimg_fea, dtype=np.float32).astype(np.float16)
    tgt = np.asarray(targetimg_fea, dtype=np.float32).astype(np.float16)
    in_maps = []
    for core in range(N_CORES):
        b, hh = core // 2, core % 2
        ref_s = ref[b, :, HP * hh:HP * (hh + 1), :].reshape(C, PIX)
        tgt_s = tgt[b, :, HP * hh:HP * (hh + 1), :].reshape(C, PIX)
        in_maps.append({"inq": _prep_core(ref_s, tgt_s), "cst": cst})
    res = run_bass_kernel_spmd(nc, in_maps, core_ids=list(range(N_CORES)),
                               trace=trace)
    out = np.empty((B, H, W), np.float32)
    for core in range(N_CORES):
        b, hh = core // 2, core % 2
        nd = res.results[core]["out"]          # [8, 1600]: den rows 0-3, num 4-7
        pred = nd[4:8] / nd[0:4]               # [4, 1600]
        out[b, HP * hh:HP * (hh + 1), :] = pred.reshape(HP, W)
    return out, res


def kernel(refimg_fea, targetimg_fea, maxdisp):
    assert int(maxdisp) == D, f"kernel hardcodes maxdisp={D}, got {maxdisp}"
    out, _ = _run(refimg_fea, targetimg_fea)
    return out


# revision 68
# speedup vs baseline: 1.0100x; 1.0100x over previous
"""HSMNet cost-volume + disparity softmax-regression on 8 Trainium2 NeuronCores.

Reference computation (per batch b):
  cost[c,d,h,w] = |ref[c,h,w] - tgt[c,h,w-d]| for w>=d else 0
  cost_agg[d,h,w] = sum_c cost
  pred[h,w] = sum_d d * softmax_d(cost_agg)

Sharding: 8 cores = 4 batches x 2 h-halves (40 rows of 80 each). Each core
processes its [32, 40, 160] slice fully fused on-chip.

Host prep (layout only, no arithmetic): inputs are cast to fp16 and
replicated into 4 partition groups (partition = c + 32*j) with the shift j
baked into tgt via a 24-col front zero pad. On-chip, per eighth of the
pixel range (800 pixels):
  - one DVE tensor_tensor subtract with a 3D access pattern (disparity
    block dim stride +4 on tgt, stride 0 broadcast on ref) produces diffs
    for all 24 disparities: diff[c+32j, k, p] = ref[c,p] - tgt[c, p-4b-j],
    b = 5-k.
  - abs in place, split across DVE (uint16 bitand), ACT (Abs), GPSIMD
    (uint16 bitand) per env-tunable column split.
  - TensorE reduces channels with 0/1 weights into PSUM [24, 2x512], plus
    one extra accumulation matmul that adds -10000 where w < d (validity
    mask folded into the PE pass: [w<d] = sum_k [k<d]*[w==k]).
  - ACT Exp evacuates PSUM -> E[96, 1600] bf16 (rows 24q+d).
  - TensorE contracts E with [ones; d] weights -> den/num [8, 1600].
  - host divides num/den (invalid entries' terms vanish: exp(-1e4) = 0).
"""
import os
import sys
import threading

for _p in ("/opt/trn_rl_repo",):
    if os.path.isdir(_p) and _p not in sys.path:
        sys.path.insert(0, _p)

import numpy as np
import ml_dtypes

import concourse.bacc as bacc
import concourse.mybir as mybir
from concourse.tile import TileContext
from concourse.bass_utils import run_bass_kernel_spmd

dt = mybir.dt

# problem shape (hardcoded per spec)
B, C, H, W = 4, 32, 80, 160
D = 24
HP = H // 2            # rows per core
PIX = HP * W           # 6400 pixels per core
NB = D // 4            # 6 disparity blocks of 4
PAD = 24               # zero pad columns in front of tgtr
NE = 8                 # processing units (eighths of the pixel range)
EW = PIX // NE         # 800 pixels per eighth
QW = PIX // 4          # 1600 pixels per quarter (E column range)
N_CORES = 8

# abs column split within each [128, 6*w] diff tile: [0:A) DVE bitand,
# [A:) ACT Abs (scaled by unit width). GPSIMD compute is NOT used: it
# shares an SBUF port with the DVE and degrades DVE throughput ~20%.
ABS_DVE = int(os.environ.get("HSM_ABS_DVE", "2496"))
DIFF_BUFS = int(os.environ.get("HSM_DIFF_BUFS", "5"))
COST_BUFS = int(os.environ.get("HSM_COST_BUFS", "2"))

# processing units (pixel offset, width): sixteenths at the start (early
# first subtract off a small first DMA) and at the end (short final
# drain abs->PE->exp->nd->copy->out); eighths in the middle. The last
# TAIL_FULL units run abs fully on DVE so ACT owes nothing at the end.
UNITS = [(0, 400), (400, 400)] + \
        [(EW * e, EW) for e in range(1, NE - 1)] + \
        [(EW * (NE - 1), 400), (EW * (NE - 1) + 400, 400)]
NU = len(UNITS)
TAIL_FULL = int(os.environ.get("HSM_TAIL_FULL", "2"))
# per-unit packed input block: [ref_w | tgt_(PAD+w)], width 2w+PAD
BLK_OFF = []
_o = 0
for (_p0, _w) in UNITS:
    BLK_OFF.append(_o)
    _o += 2 * _w + PAD
IN_TOTAL = _o


# packed consts (all fp16 container): lred | lnd(bf16 bits) | lmask | maskc
CONST_W = NB * D + 8 + D + EW  # 144+8+24+800 = 976


def _build_program():
    nc = bacc.Bacc("TRN2", target_bir_lowering=False)
    inq_h = nc.dram_tensor("inq", [128, IN_TOTAL], dt.float16,
                           kind="ExternalInput")
    cst_h = nc.dram_tensor("cst", [128, CONST_W], dt.float16,
                           kind="ExternalInput")
    out_h = nc.dram_tensor("out", [8, 4 * 400], dt.float32, kind="ExternalOutput")

    with TileContext(nc) as tc:
        with tc.tile_pool(name="const", bufs=1) as cpool, \
             tc.tile_pool(name="inp", bufs=6) as ipool, \
             tc.tile_pool(name="diffp", bufs=DIFF_BUFS) as dpool, \
             tc.tile_pool(name="ep", bufs=1) as epool:
            cst_sb = cpool.tile([128, CONST_W], dt.float16)
            lred_sb = cst_sb[:, 0:NB * D]
            lnd_sb = cst_sb[:, NB * D:NB * D + 8].bitcast(dt.bfloat16)
            lmask_sb = cst_sb[0:D, NB * D + 8:NB * D + 8 + D]
            maskc_sb = cst_sb[0:D, NB * D + 8 + D:CONST_W]

            E = epool.tile([128, QW], dt.bfloat16)

            # one packed DMA per unit (ref_w | shifted tgt window), spread
            # over the sync/scalar trigger queues so the ~0.6us
            # descriptor-gen per DMA doesn't serialize ahead of unit 0
            inq = {}

            def emit_load(u, eng):
                _, w = UNITS[u]
                bw = 2 * w + PAD
                t_ = ipool.tile([128, 2 * EW + PAD], dt.float16, tag="inq",
                                name=f"inq_{u}", bufs=6)
                eng.dma_start(t_[:, 0:bw],
                              inq_h[:, BLK_OFF[u]:BLK_OFF[u] + bw])
                inq[u] = t_

            emit_load(0, nc.sync)
            emit_load(1, nc.scalar)
            emit_load(2, nc.sync)
            emit_load(3, nc.scalar)
            nc.sync.dma_start(cst_sb[:], cst_h[:])
            # rows 32q+24..32q+31 are never written by the exp evac; zero
            # them once so the num/den matmul sees 0 (their lnd weights are
            # 0, but garbage could be inf/nan)
            nc.gpsimd.memset(E[:], 0.0)
            for u in range(4, NU):
                emit_load(u, nc.sync if u % 2 == 0 else nc.scalar)

            diffs = {}

            def emit_tt(u):
                _, w = UNITS[u]
                diff = dpool.tile([128, NB * EW], dt.float16, tag="diff",
                                  name=f"diff_{u}", bufs=DIFF_BUFS)
                out = diff[:, 0:NB * w].rearrange("p (k x) -> p k x", x=w)
                in0 = inq[u][:, 0:w].unsqueeze(1).broadcast_to(
                    [128, NB, w])
                # tgt part starts at col w; block k reads window cols
                # [w+4+4k, +w) -> diff slot k holds block b = 5-k
                in1 = inq[u][:, w + 4:w + 4 + w].unsqueeze(
                    1).broadcast_to([128, NB, w]).copy()
                in1.ap = in1.ap[:1] + (((4, NB)),) + in1.ap[2:]
                nc.vector.tensor_tensor(out, in0, in1,
                                        mybir.AluOpType.subtract)
                diffs[u] = diff
                # tail units have no ACT abs; run their (full-width) DVE
                # bitand right away so their PE/exp overlap the next TT
                if u >= NU - TAIL_FULL:
                    emit_abs_dve(u)
                    abs_done.add(u)

            def _asplit(u):
                w = UNITS[u][1]
                if u >= NU - TAIL_FULL:
                    return NB * w
                return ABS_DVE * w // EW

            abs_done = set()

            def emit_abs_dve(u):
                if u in abs_done:
                    return
                a = _asplit(u)
                if a > 0:
                    du = diffs[u][:, 0:a].bitcast(dt.uint16)
                    nc.vector.tensor_scalar(du, du, 0x7FFF, None,
                                            mybir.AluOpType.bitwise_and)

            def emit_abs_act(u):
                a, w = _asplit(u), UNITS[u][1]
                if a < NB * w:
                    nc.scalar.activation(diffs[u][:, a:NB * w],
                                         diffs[u][:, a:NB * w],
                                         mybir.ActivationFunctionType.Abs)

            costs = {}

            def emit_pe(u, qpool):
                p0, w = UNITS[u]
                diff = diffs[u]
                ccw = min(400, w)
                ncc = w // ccw
                cost = qpool.tile([D, 1024], dt.float32, tag="cost",
                                  name=f"cost_{u}")
                for k in range(NB):
                    b = NB - 1 - k
                    for cc in range(ncc):
                        nc.tensor.matmul(
                            cost[:, 512 * cc:512 * cc + ccw],
                            lred_sb[:, D * b:D * (b + 1)],
                            diff[:, w * k + ccw * cc:w * k + ccw * cc + ccw],
                            start=(k == 0), stop=False)
                for cc in range(ncc):
                    mo = (p0 + ccw * cc) % EW
                    nc.tensor.matmul(
                        cost[:, 512 * cc:512 * cc + ccw],
                        lmask_sb[:],
                        maskc_sb[:, mo:mo + ccw],
                        start=False, stop=(cc == ncc - 1))
                costs[u] = cost

            def emit_exp(u):
                p0, w = UNITS[u]
                q, ecol = p0 // QW, p0 % QW
                ccw = min(400, w)
                ncc = w // ccw
                cost = costs[u]
                src = cost[:, 0:512 * ncc].rearrange(
                    "p (k x) -> p k x", x=512)[:, :, 0:ccw]
                dst = E[32 * q:32 * q + D, ecol:ecol + w].rearrange(
                    "p (k x) -> p k x", x=ccw)
                nc.scalar.activation(dst, src, mybir.ActivationFunctionType.Exp)
                del costs[u], diffs[u]

            with tc.tile_pool(name="cost", bufs=COST_BUFS, space="PSUM") as qpool, \
                 tc.tile_pool(name="nd", bufs=1, space="PSUM") as npool:
                nd = npool.tile([8, 2048], dt.float32)

                # HAM warm-up: keep the PE clock at full rate until the
                # first real matmul burst (reads zeroed E, overwritten by
                # the real nd accumulation later). Must bridge the whole
                # window to the first real matmul (~15us) or HAM
                # re-throttles after ~3.4us idle.
                for _ in range(8):
                    nc.tensor.matmul(nd[:, 0:400], E[:, 0:8], E[:, 0:400],
                                     start=True, stop=True)

                def emit_nd(half):
                    # partial num/den contraction over quarters 2h, 2h+1
                    # (PE operands cannot start at partition 96, so halves)
                    for cc in range(4):
                        nc.tensor.matmul(
                            nd[:, 512 * cc:512 * cc + 400],
                            lnd_sb[64 * half:64 * half + 64, :],
                            E[64 * half:64 * half + 64,
                              400 * cc:400 * (cc + 1)],
                            start=(half == 0), stop=(half == 1))

                for u in range(NU):
                    emit_tt(u)
                    emit_abs_act(u)
                    if u >= 1:
                        emit_abs_dve(u - 1)
                        emit_pe(u - 1, qpool)
                        emit_exp(u - 1)
                        if u == 5:
                            emit_nd(0)
                emit_abs_dve(NU - 1)
                emit_pe(NU - 1, qpool)
                emit_exp(NU - 1)
                emit_nd(1)

                # evacuate nd per 512-chunk as each region's accumulation
                # completes, split across DVE (cc0/1) and ACT (cc2/3)
                ndv = nd[:].rearrange("p (k x) -> p k x", x=512)[:, :, 0:400]
                out_a = epool.tile([8, 800], dt.float32)
                out_b = epool.tile([8, 800], dt.float32)
                nc.vector.tensor_copy(
                    out_a[:].rearrange("p (k x) -> p k x", x=400),
                    ndv[:, 0:2])
                nc.scalar.activation(
                    out_b[:].rearrange("p (k x) -> p k x", x=400),
                    ndv[:, 2:4], mybir.ActivationFunctionType.Copy)
                nc.sync.dma_start(out_h[:, 0:800], out_a[:])
                nc.scalar.dma_start(out_h[:, 800:1600], out_b[:])

    nc.compile()
    return nc


def _host_constants():
    """Single packed [128, CONST_W] fp16 tensor: lred | lnd bits | lmask | maskc."""
    cst = np.zeros((128, CONST_W), np.float16)
    # lred: block b sums channels of partition group j into cost row 4b+j
    for b in range(NB):
        for j in range(4):
            for c in range(C):
                cst[c + 32 * j, D * b + 4 * b + j] = 1.0

    # lnd (bf16, stored as raw bits in the fp16 container)
    lnd = np.zeros((128, 8), np.float32)
    for q in range(4):
        for d in range(D):
            lnd[d + 32 * q, q] = 1.0      # den
            lnd[d + 32 * q, 4 + q] = d    # num
    lnd16 = lnd.astype(ml_dtypes.bfloat16).view(np.float16)
    cst[:, NB * D:NB * D + 8] = lnd16

    # bias[d, p] = sum_k lmask[k, d] * maskc[k, p] = -1e4 * [(p mod W) < d]
    o = NB * D + 8
    for k in range(D):
        for d in range(D):
            if k < d:
                cst[k, o + d] = 1.0
    o = NB * D + 8 + D
    for k in range(D):
        cst[k, o + np.arange(EW)[np.arange(EW) % W == k]] = -10000.0
    return cst


_lock = threading.Lock()
_cache = {}


def _get_program():
    with _lock:
        if "nc" not in _cache:
            _cache["nc"] = _build_program()
            _cache["consts"] = _host_constants()
        return _cache["nc"], _cache["consts"]


def _prep_core(ref_s, tgt_s):
    """ref_s, tgt_s: [32, 6400] fp16 -> packed [128, IN_TOTAL] input."""
    inq = np.zeros((128, IN_TOTAL), np.float16)
    for u, (c0, w) in enumerate(UNITS):
        o = BLK_OFF[u]
        inq[:, o:o + w] = np.broadcast_to(
            ref_s[None, :, c0:c0 + w], (4, C, w)).reshape(128, w)
        # tgt window [c0 - PAD, c0 + w) with per-group shift j baked in:
        # col o + w + s holds tgt[c, c0 - PAD + s - j]
        for j in range(4):
            lo = c0 - PAD - j
            src = tgt_s[:, max(lo, 0):c0 + w - j]
            d0 = o + w + max(lo, 0) - lo
            inq[32 * j:32 * j + 32, d0:d0 + src.shape[1]] = src
    return inq


def _run(refimg_fea, targetimg_fea, trace=False):
    nc, cst = _get_program()
    ref = np.asarray(refimg_fea, dtype=np.float32).astype(np.float16)
    tgt = np.asarray(targetimg_fea, dtype=np.float32).astype(np.float16)
    in_maps = []
    for core in range(N_CORES):
        b, hh = core // 2, core % 2
        ref_s = ref[b, :, HP * hh:HP * (hh + 1), :].reshape(C, PIX)
        tgt_s = tgt[b, :, HP * hh:HP * (hh + 1), :].reshape(C, PIX)
        in_maps.append({"inq": _prep_core(ref_s, tgt_s), "cst": cst})
    res = run_bass_kernel_spmd(nc, in_maps, core_ids=list(range(N_CORES)),
                               trace=trace)
    out = np.empty((B, H, W), np.float32)
    for core in range(N_CORES):
        b, hh = core // 2, core % 2
        nd = res.results[core]["out"]          # [8, 1600]: den rows 0-3, num 4-7
        pred = nd[4:8] / nd[0:4]               # [4, 1600]
        out[b, HP * hh:HP * (hh + 1), :] = pred.reshape(HP, W)
    return out, res


def kernel(refimg_fea, targetimg_fea, maxdisp):
    assert int(maxdisp) == D, f"kernel hardcodes maxdisp={D}, got {maxdisp}"
    out, _ = _run(refimg_fea, targetimg_fea)
    return out
